# revision 1
# baseline (speedup 1.0000x reference)
"""Trainium2 Bass kernel for nn_InverseRecurrentLayer.

Reference computation:
    W_inv = inv(W)
    h[t] = inputs[:, t, :] @ R + bias      # [B, U]  (bias folded into h)
    s_{t+1} = tanh(h[t] + s_t @ Wt),  Wt = W if (t//64)%2==1 else W_inv
    output = states [T, B, U]

Shapes: B=64, T=512, F=512, U=1024. fp32. Data-parallel over batch:
8 cores x B_loc=8.

Per-core plan (v4 — PE-saturated scan, 1.937ms/core in TimelineSim vs
3.23ms for the v1 baseline; PE 94.8% busy):
  Phase A: h = xT.T @ [R; bias] computed as [tb, u] tiles and kept RESIDENT
  in SBUF as bf16 (h_sb, 8MB; no DRAM round-trip). Emitted as a work-item
  stream: 2 h-blocks bootstrap up-front, the remaining matmuls interleave
  one-per-step into the scan's PE bubble so the projection is hidden.

  Phase B: 512-step scan, 4 u-quarters (256 wide) per step:
    - 8 accumulating matmuls per quarter (stationary = sT chunk [128,8],
      moving = W rows [128,256], fp32r => 1 cycle/col; the W stream is the
      hard floor: 8192 cols/step = 3.4us),
    - DVE add: z = ps + h_step (h staged per step to partitions 0..7 by a
      prefetched SBUF-to-SBUF DMA; compute engines need 32-aligned
      partition bases so they cannot read h_sb rows directly),
    - PE transpose of z chunks [8,128] -> ptr [128,8] (identity matmul),
    - ACT tanh(ptr) -> sT quarter tile (f32r, transposed state, no copy).
  Every per-step tensor is a per-quarter tile (whole-tile WAR hazards
  otherwise serialize transposes behind unrelated tanhs); ptr PSUM tiles
  are shared by lifetime-disjoint quarter pairs to fit 8 PSUM banks; the
  emission order software-pipelines each quarter's add/tr/tanh tail under
  later matmul groups so the PE never idles (pstate stays at 2.4 GHz).
  Output is stored transposed [t, 128, 64]; the host driver untransposes.

This environment's walrus encodes at most ONE sync-wait command per
instruction; legalize_waits() hoists extra waits onto InstNoOp carriers,
and the Tile exit barrier is patched to sem-only barriers.
"""
import sys

sys.path.insert(0, "/opt/trn_rl_repo")

import numpy as np
from contextlib import ExitStack

import concourse.bass as bass
import concourse.mybir as mybir
import concourse.tile as tile
from concourse.bass_utils import run_bass_kernel_spmd

# ---------------------------------------------------------------- constants
B, T, F, U = 64, 512, 512, 1024
NCORES = 8
BLOC = B // NCORES          # 8 batch rows per core
KF = F // 128               # 4 k-tiles for the projection
KU = U // 128               # 8 k-tiles (state chunks) for the scan
NQ = 4                      # u-quarters in the scan
QW = U // NQ                # 256 quarter width
F32 = mybir.dt.float32
F32R = mybir.dt.float32r
BF16 = mybir.dt.bfloat16

# ------------------------------------------------- walrus wait legalization


def _patched_drain_and_barrier(self, tick_clock, wait_clock):
    drain_inst = self.nc.sync.drain()
    wait_clock.add_sem_waits(
        drain_inst.ins, tile.ScopedClock({None: tick_clock.global_clock})
    )
    ow = list(drain_inst.ins.sync_info.on_wait or [])
    if len(ow) > 1:
        drain_inst.ins.sync_info.on_wait = ow[:1]
        for w in ow[1:]:
            d2 = self.nc.sync.drain()
            d2.ins.sync_info = mybir.SyncInfo(on_wait=[w], on_update=[])
    self.nc.all_engine_barrier(sem_only=True)
    popped = self.nc._tile_sem_poison_stack.pop()
    assert popped is self._sem_poison
    self.nc.clear_and_free_semaphores(list(self.sems.allocated().values()))
    self.nc.all_engine_barrier(sem_only=True)


tile.TileContext._drain_and_barrier = _patched_drain_and_barrier


def legalize_waits(nc):
    """Split multi-wait instructions: keep 1 wait, hoist the rest onto
    InstNoOp carriers inserted just before, on the same engine."""
    n = 0
    for fn in nc.m.functions:
        for blk in fn.blocks:
            out = []
            for inst in blk.instructions:
                si = inst.sync_info
                if si is not None and si.on_wait and len(si.on_wait) > 1:
                    waits = list(si.on_wait)
                    for w in waits[:-1]:
                        n += 1
                        nop = mybir.InstNoOp(
                            name=f"waitcar-{n}-{inst.name}",
                            engine=inst.engine,
                            ins=[],
                            outs=[],
                            sync_info=mybir.SyncInfo(on_wait=[w], on_update=[]),
                        )
                        nc.register_instruction(nop)
                        out.append(nop)
                    si.on_wait = waits[-1:]
                out.append(inst)
            blk.instructions[:] = out
    return n


# ------------------------------------------------------------ device kernel

LABELS = {}


def _lbl(bi, label):
    try:
        LABELS[bi.ins.name] = label
    except Exception:
        pass
    return bi


def build_kernel(t_steps=T, with_bias=True):
    nc = bass.Bass("TRN2", target_bir_lowering=False, debug=False)
    tb = BLOC * t_steps
    m_tiles = tb // 128          # 128-col tb tiles in phase A
    h_blocks = m_tiles           # h_sb u-blocks, one per 16 steps

    xT_d = nc.dram_tensor("xT", [F, tb], F32R, kind="ExternalInput").ap()
    Ra_d = nc.dram_tensor("Ra", [F + 1, U], F32R, kind="ExternalInput").ap()
    W2_d = nc.dram_tensor("W2", [2, U, U], F32R, kind="ExternalInput").ap()
    x0T_d = nc.dram_tensor("x0T", [128, KU], F32, kind="ExternalInput").ap()
    id8_d = nc.dram_tensor("id8", [8, 8], F32, kind="ExternalInput").ap()
    out_d = nc.dram_tensor("out", [t_steps, 128, KU * BLOC], F32,
                           kind="ExternalOutput").ap()

    with tile.TileContext(nc) as tc, ExitStack() as ctx:
        const = ctx.enter_context(tc.tile_pool(name="const", bufs=1))
        # resident tensors
        w_sb = const.tile([128, 2 * KU * 1024], F32R)      # 8 MB, both phases
        ra_sb = const.tile([128, KF * 1024], F32R)         # R k-tiles
        rb_sb = const.tile([1, 1024], F32R)                # bias row of Ra
        h_sb = const.tile([128, h_blocks * 1024], BF16)    # resident h, 8 MB
        xo_sb = const.tile([1, 128], F32R)                 # ones row (const)
        x0_sb = const.tile([128, KU], F32)
        id8 = const.tile([8, 8], F32)

        # phase-A inputs first so the W bulk doesn't hog the DMA queues
        # during bootstrap
        for k in range(KF):
            nc.sync.dma_start(
                ra_sb[:, k * 1024:(k + 1) * 1024], Ra_d[k * 128:(k + 1) * 128, :]
            )
        nc.sync.dma_start(rb_sb[:, :], Ra_d[F:F + 1, :])
        nc.sync.dma_start(x0_sb[:, :], x0T_d[:, :])
        nc.sync.dma_start(id8[:, :], id8_d[:, :])
        for p in range(2):
            for k in range(KU):
                nc.sync.dma_start(
                    w_sb[:, (p * KU + k) * 1024:(p * KU + k + 1) * 1024],
                    W2_d[p, k * 128:(k + 1) * 128, :],
                )
        # constant ones row for the bias matmul
        ones_f = xo_sb.bitcast(F32)
        nc.vector.memset(ones_f, 1.0)

        # ---------------- phase A: h = xT.T @ [R; bias] -> bf16 SBUF.
        # Structured as a work-item generator: the first blocks are emitted
        # up-front, the rest interleave into the scan's per-step PE bubble
        # (one matmul per step) so the projection costs no extra wall time.
        MQ = min(4, m_tiles)      # m-tiles fetched per DMA batch
        xpool = ctx.enter_context(tc.tile_pool(name="xstage", bufs=2))
        papool = ctx.enter_context(
            tc.tile_pool(name="psum_proj", bufs=1, space="PSUM")
        )
        pa_state = {}

        def phase_a_items():
            for mq in range(m_tiles // MQ):
                def dma_item(mq=mq):
                    xa = xpool.tile([128, KF, MQ * 128], F32R, tag="xa",
                                    name="xa")
                    for k in range(KF):
                        # off the SP queue (it carries the out/h DMAs)
                        nc.scalar.dma_start(
                            xa[:, k, :],
                            xT_d[k * 128:(k + 1) * 128,
                                 mq * MQ * 128:(mq + 1) * MQ * 128],
                        )
                    pa_state["xa"] = xa
                yield ("dma", dma_item)
                for j in range(MQ):
                    m = mq * MQ + j
                    for n in range(2):
                        n_mms = KF + 1 if with_bias else KF
                        for k in range(n_mms):
                            def mm_item(j=j, n=n, k=k, last=(k == n_mms - 1)):
                                if k == 0:
                                    pa_state["ps"] = papool.tile(
                                        [128, 512], F32, tag="psA", name="psA"
                                    )
                                if k < KF:
                                    nc.tensor.matmul(
                                        pa_state["ps"][:],
                                        pa_state["xa"][:, k,
                                                       j * 128:(j + 1) * 128],
                                        ra_sb[:, k * 1024 + n * 512:
                                              k * 1024 + n * 512 + 512],
                                        start=(k == 0),
                                        stop=last,
                                    )
                                else:
                                    nc.tensor.matmul(
                                        pa_state["ps"][:],
                                        xo_sb[:, :],
                                        rb_sb[:, n * 512:n * 512 + 512],
                                        start=False,
                                        stop=True,
                                    )
                            yield ("mm", mm_item)

                        def copy_item(m=m, n=n):
                            # Pool/GPSIMD cannot read PSUM on HW; DVE has an
                            # idle window after the step's four adds, ACT
                            # copies would delay the carried tanh q2
                            dst = h_sb[:, m * 1024 + n * 512:
                                       m * 1024 + n * 512 + 512]
                            nc.vector.tensor_copy(dst, pa_state["ps"][:])
                        yield ("copy", copy_item)

        pa_iter = phase_a_items()
        pa_next = [next(pa_iter)]

        def pa_pull(kinds, limit=1):
            done = 0
            while done < limit and pa_next[0] is not None:
                kind, fn = pa_next[0]
                if kind not in kinds:
                    return
                fn()
                done += 1
                pa_next[0] = next(pa_iter, None)

        # bootstrap: h blocks 0..1 (1 dma batch + 2 blocks of items)
        blk_items = 2 * ((KF + 1 if with_bias else KF) + 1)
        pa_pull(("dma", "mm", "copy"), limit=1 + 2 * blk_items)

        # ---------------- phase B: the scan
        # Every per-step tensor is split per u-quarter into its own tile:
        # the tile framework treats write-after-read hazards at whole-tile
        # granularity, and shared tiles serialize transposes behind
        # unrelated tanhs. PSUM budget: 4 mm tiles + 4 ptr tiles = 8 banks.
        mmp = ctx.enter_context(tc.tile_pool(name="psum_mm", bufs=1, space="PSUM"))
        trp = ctx.enter_context(tc.tile_pool(name="psum_tr", bufs=1, space="PSUM"))
        zpool = ctx.enter_context(tc.tile_pool(name="z", bufs=3))
        spool = ctx.enter_context(tc.tile_pool(name="sT", bufs=3))
        hpool = ctx.enter_context(tc.tile_pool(name="hstep", bufs=3))
        QC = 2 * BLOC                                    # 16 cols per quarter

        # initial transposed state from x0 (per quarter)
        sT_prev = []
        for q in range(NQ):
            sq = spool.tile([128, QC], F32R, tag=f"sT{q}", name=f"sTi{q}")
            for c in range(2):
                nc.vector.tensor_copy(
                    sq[:, c * BLOC:(c + 1) * BLOC],
                    x0_sb[:, 2 * q + c:2 * q + c + 1].broadcast_to([128, BLOC]),
                )
            sT_prev.append(sq)

        ps_t = [None] * NQ          # live psum tiles per quarter
        z_t = [None] * NQ           # live z tiles per quarter
        ptr_t = [None] * NQ         # live transposed-preact tiles per quarter
        sT_of = {-1: sT_prev}       # step -> [4 quarter state tiles]

        def emit_mms(t, q, ks, sT_src):
            p = 1 if (t // 64) % 2 == 1 else 0
            for k in ks:
                if k == 0:
                    ps_t[q] = mmp.tile([BLOC, QW], F32, tag=f"mm{q}",
                                       name=f"ps{q}",
                                       bufs=2 if q == 3 else 1)
                wc = (p * KU + k) * 1024 + q * QW
                sq = sT_src[k // 2]
                _lbl(nc.tensor.matmul(
                    ps_t[q][:],
                    sq[:, (k % 2) * BLOC:(k % 2 + 1) * BLOC],
                    w_sb[:, wc:wc + QW],
                    start=(k == 0),
                    stop=(k == KU - 1),
                ), f"mm t{t} q{q} k{k}")

        h_step = {}                 # step -> staged [8, 1024] h tile

        def emit_hprefetch(t):
            # compute engines need 32-aligned partition bases, so each step's
            # h rows are DMA-staged (partition-free) to partitions 0..7 one
            # step ahead of use
            if t >= t_steps:
                return
            ht = hpool.tile([BLOC, U], BF16, tag="hstep", name=f"hs{t % 3}")
            prow = (t % 16) * 8
            blk = t // 16
            nc.sync.dma_start(
                ht[:], h_sb[prow:prow + 8, blk * 1024:(blk + 1) * 1024]
            )
            h_step[t] = ht
            h_step.pop(t - 3, None)

        def emit_add(t, q, engine, split=False):
            z_t[q] = zpool.tile([BLOC, QW], F32, tag=f"z{q}", name=f"z{q}")
            if split:
                # two half-adds so the first transpose can start ~140ns
                # earlier (chunk 2q's chain is the cycle-critical path)
                for h in range(2):
                    _lbl(engine.tensor_add(
                        z_t[q][:, h * 128:(h + 1) * 128],
                        ps_t[q][:, h * 128:(h + 1) * 128],
                        h_step[t][:, q * QW + h * 128:q * QW + (h + 1) * 128],
                    ), f"add t{t} q{q}{'ab'[h]}")
            else:
                hsl = h_step[t][:, q * QW:(q + 1) * QW]
                _lbl(engine.tensor_add(z_t[q][:], ps_t[q][:], hsl),
                     f"add t{t} q{q}")

        # ptr PSUM tiles are shared by quarter pairs with disjoint lifetime
        # windows — q0+q3 ("Y") and q1+q2 ("X") — to fit 8 PSUM banks while
        # double-buffering the q2/q3 matmul tiles.
        ptr_pair = {"X": {}, "Y": {}}
        PAIR = {0: ("Y", 0), 3: ("Y", QC), 1: ("X", 0), 2: ("X", QC)}

        def emit_chain(t, q, split=False, trs_only=False, tanh_only=False):
            # transposes + tanh + store for quarter q of step t
            pk, po = PAIR[q]
            if po == 0:
                ptr_pair[pk][t] = trp.tile([128, 2 * QC], F32, tag=f"ptr{pk}",
                                           name=f"ptr{pk}")
                ptr_pair[pk].pop(t - 2, None)
            ptr = ptr_pair[pk][t]
            if not tanh_only:
                for half in range(2):
                    _lbl(nc.tensor.transpose(
                        ptr[:, po + half * BLOC:po + (half + 1) * BLOC],
                        z_t[q][:, half * 128:(half + 1) * 128],
                        id8[:, :],
                    ), f"tr t{t} c{2 * q + half}")
            if trs_only:
                return
            sq = spool.tile([128, QC], F32R, tag=f"sT{q}", name=f"sT{q}")
            sT_of[t][q] = sq
            for half in range(2):
                if split:
                    _lbl(nc.scalar.activation(
                        sq[:, half * BLOC:(half + 1) * BLOC],
                        ptr[:, po + half * BLOC:po + (half + 1) * BLOC],
                        mybir.ActivationFunctionType.Tanh,
                    ), f"tanh t{t} q{q}{'ab'[half]}")
            if not split:
                _lbl(nc.scalar.activation(
                    sq[:, :],
                    ptr[:, po:po + QC],
                    mybir.ActivationFunctionType.Tanh,
                ), f"tanh t{t} q{q}")
            nc.sync.dma_start(out_d[t, :, q * QC:(q + 1) * QC],
                              sq[:].bitcast(F32))

        emit_hprefetch(0)
        emit_hprefetch(1)
        for t in range(t_steps):
            sT_of[t] = [None] * NQ
            src = sT_of[t - 1]
            emit_hprefetch(t + 2)
            # src[c//2]: quarter tiles of step t-1's state; quarters 2,3 are
            # finished below (emit_chain(t-1, 2/3)) before any k4..7 matmul
            # consumes them.
            for q in range(NQ):
                emit_mms(t, q, (0, 1), src)              # all k0,k1
            if t > 0:
                emit_chain(t - 1, 2)                     # trs c4,c5 + tanh
                emit_chain(t - 1, 3)                     # trs c6,c7 + tanh
            for q in range(3):
                emit_mms(t, q, (2, 3), src)              # q0..q2 k2,k3
            emit_mms(t, 0, (4, 5, 6, 7), src)            # q0[k4..7]
            emit_add(t, 0, nc.vector)                    # add q0 (DVE)
            emit_mms(t, 3, (2, 3), src)                  # q3[k2,3] deferred
            emit_mms(t, 1, (4, 5, 6, 7), src)            # q1[k4..7]
            emit_add(t, 1, nc.vector)                    # add q1 (DVE)
            emit_mms(t, 2, (4, 5), src)                  # q2[k4,5]
            emit_chain(t, 0)                             # trs c0,c1 + tanh q0
            emit_mms(t, 2, (6, 7), src)                  # q2[k6,7]
            emit_add(t, 2, nc.vector)                    # add q2 (DVE)
            emit_mms(t, 3, (4, 5, 6), src)               # q3[k4..6]
            emit_chain(t, 1)                             # trs c2,c3 + tanh q1
            emit_mms(t, 3, (7,), src)                    # q3[k7]
            emit_add(t, 3, nc.vector)                    # add q3 (DVE)
            pa_pull(("mm",), limit=1)                    # filler mm at the
                                                         # cycle boundary
            pa_pull(("copy", "dma"), limit=2)            # phase-A non-PE items
            sT_of.pop(t - 2, None)

        # epilogue: finish last step's quarters 2,3
        tl = t_steps - 1
        emit_chain(tl, 2)
        emit_chain(tl, 3)

    legalize_waits(nc)
    return nc


# -------------------------------------------------------------- host driver
_CACHE = {}


def _get_nc(t_steps, with_bias=True):
    key = (t_steps, with_bias)
    if key not in _CACHE:
        _CACHE[key] = build_kernel(t_steps, with_bias)
    return _CACHE[key]


def kernel(inputs, R, W, bias, x0, t_steps=None, n_cores=NCORES, trace=False,
           trace_kw=None):
    t_steps = t_steps or inputs.shape[1]
    inputs = np.ascontiguousarray(inputs, dtype=np.float32)
    R = np.asarray(R, dtype=np.float32)
    W = np.asarray(W, dtype=np.float32)
    bias = np.asarray(bias, dtype=np.float32)
    x0 = np.asarray(x0, dtype=np.float32)

    W_inv = np.linalg.inv(W)
    W2 = np.stack([W_inv, W]).astype(np.float32)        # phase 0 = W_inv
    Ra = np.concatenate([R, bias[None, :]], axis=0)      # [F+1, U]
    x0T = np.ascontiguousarray(x0.reshape(KU, 128).T)    # [128, KU]
    id8 = np.eye(8, dtype=np.float32)

    in_maps = []
    for c in range(n_cores):
        xc = inputs[c * BLOC:(c + 1) * BLOC, :t_steps, :]   # [BLOC, t, F]
        # xT[f, t*BLOC+b] (t-major cols)
        xT = np.ascontiguousarray(
            xc.transpose(2, 1, 0).reshape(F, BLOC * t_steps)
        )
        in_maps.append(
            {"xT": xT, "Ra": Ra, "W2": W2, "x0T": x0T, "id8": id8}
        )

    nc = _get_nc(t_steps, with_bias=bool(np.any(bias)))
    try:
        res = run_bass_kernel_spmd(
            nc, in_maps, core_ids=list(range(n_cores)), trace=trace,
            **(trace_kw or {}),
        )
    except Exception:
        # transient device wedges (NRT_EXEC_UNIT_UNRECOVERABLE) usually
        # clear on a retry
        res = run_bass_kernel_spmd(
            nc, in_maps, core_ids=list(range(n_cores)), trace=trace,
            **(trace_kw or {}),
        )
    kernel.last_result = res
    kernel.last_nc = nc
    # assemble [T, B, U]: per-core out is [t, 128, KU*BLOC] transposed state
    full = np.empty((t_steps, n_cores * BLOC, U), np.float32)
    for c in range(n_cores):
        arr = res.results[c]["out"]                      # [t, 128, 64]
        full[:, c * BLOC:(c + 1) * BLOC, :] = (
            arr.reshape(t_steps, 128, KU, BLOC)
            .transpose(0, 3, 2, 1)
            .reshape(t_steps, BLOC, U)
        )
    return full



# revision 7
# speedup vs baseline: 3.7823x; 3.7823x over previous
"""Trainium2 Bass kernel for nn_InverseRecurrentLayer.

Reference computation:
    W_inv = inv(W)
    h[t] = inputs[:, t, :] @ R + bias      # [B, U]  (bias folded into h)
    s_{t+1} = tanh(h[t] + s_t @ Wt),  Wt = W if (t//64)%2==1 else W_inv
    output = states [T, B, U]

Shapes: B=64, T=512, F=512, U=1024. fp32 in/out. Data-parallel over
batch: 8 cores x B_loc=8.

Per-core plan (v5 — W-stationary scan):
  The v4 kernel streamed all of W through the PE as the *moving* tensor
  every step (8192 moving cols/step -> 3.4us/step floor). v5 flips the
  matmul: W chunks [128,128] are the *stationary* operand and the
  transposed state sT chunk [128, 8] is the moving operand, so a full
  state update streams only 8kc x 8uo x 8 cols = 512 moving columns.
  With bf16 operands (1 cycle/col at any size) the per-step PE time is
  ~0.25us and the critical path becomes the PE->ACT->PE latency chain.

  State layout: one tile s[128, 64], col = j*8 + b for u-chunk j, batch
  b; partition = u % 128. A step is:
    - 1 identity matmul: psum[128,64]  = I128^T @ h_t[128,64] (start)
    - 64 matmuls:        psum[:, j*8:] += Wc[kc,j]^T @ s_prev[:, kc*8:]
    - 1 ACT tanh:        s_new[128,64](bf16) = tanh(psum)
  s_new is written into a 16-step ring; one DMA per 16 steps stores the
  ring to DRAM (bf16; host converts/untransposes).

  Phase A (h = x @ R + bias, transposed layout hT[u, (t,j,b)]) runs as
  bf16 matmuls (256-col moving chunks) interleaved one item per step
  into the scan's PE idle window, with DVE draining psum-> resident
  h_sb tiles (one tile per 64-step chunk to avoid whole-tile hazards
  with the scan's h reads).

This environment's walrus encodes at most ONE sync-wait command per
instruction; legalize_waits() hoists extra waits onto InstNoOp carriers,
and the Tile exit barrier is patched to sem-only barriers.
"""
import sys

sys.path.insert(0, "/opt/trn_rl_repo")

import numpy as np
from contextlib import ExitStack

import concourse.bass as bass
import concourse.mybir as mybir
import concourse.tile as tile
from concourse.bass_utils import run_bass_kernel_spmd

# ---------------------------------------------------------------- constants
B, T, F, U = 64, 512, 512, 1024
NCORES = 8
BLOC = B // NCORES          # 8 batch rows per core
KF = F // 128               # 4 f-chunks for the projection
KU = U // 128               # 8 u-chunks
TCH = 64                    # steps per h chunk (= INVERT_INDEX)
RING = 16                   # steps per output ring/DMA
F32 = mybir.dt.float32
BF16 = mybir.dt.bfloat16

# ------------------------------------------------- walrus wait legalization


def _patched_drain_and_barrier(self, tick_clock, wait_clock):
    drain_inst = self.nc.sync.drain()
    wait_clock.add_sem_waits(
        drain_inst.ins, tile.ScopedClock({None: tick_clock.global_clock})
    )
    ow = list(drain_inst.ins.sync_info.on_wait or [])
    if len(ow) > 1:
        drain_inst.ins.sync_info.on_wait = ow[:1]
        for w in ow[1:]:
            d2 = self.nc.sync.drain()
            d2.ins.sync_info = mybir.SyncInfo(on_wait=[w], on_update=[])
    self.nc.all_engine_barrier(sem_only=True)
    popped = self.nc._tile_sem_poison_stack.pop()
    assert popped is self._sem_poison
    self.nc.clear_and_free_semaphores(list(self.sems.allocated().values()))
    self.nc.all_engine_barrier(sem_only=True)


tile.TileContext._drain_and_barrier = _patched_drain_and_barrier


def legalize_waits(nc):
    """Split multi-wait instructions: keep 1 wait, hoist the rest onto
    InstNoOp carriers inserted just before, on the same engine."""
    n = 0
    for fn in nc.m.functions:
        for blk in fn.blocks:
            out = []
            for inst in blk.instructions:
                si = inst.sync_info
                if si is not None and si.on_wait and len(si.on_wait) > 1:
                    waits = list(si.on_wait)
                    for w in waits[:-1]:
                        n += 1
                        nop = mybir.InstNoOp(
                            name=f"waitcar-{n}-{inst.name}",
                            engine=inst.engine,
                            ins=[],
                            outs=[],
                            sync_info=mybir.SyncInfo(on_wait=[w], on_update=[]),
                        )
                        nc.register_instruction(nop)
                        out.append(nop)
                    si.on_wait = waits[-1:]
                out.append(inst)
            blk.instructions[:] = out
    return n


# ------------------------------------------------------------ device kernel


def build_kernel(t_steps=T, with_bias=False):
    assert t_steps % RING == 0
    nc = bass.Bass("TRN2", target_bir_lowering=False, debug=False)
    tb = BLOC * t_steps
    n_ch = (t_steps + TCH - 1) // TCH       # h chunks (64 steps each)

    xT_d = nc.dram_tensor("xT", [F, tb], BF16, kind="ExternalInput").ap()
    R_d = nc.dram_tensor("R", [F, U], BF16, kind="ExternalInput").ap()
    W2_d = nc.dram_tensor("W2", [2, U, U], BF16, kind="ExternalInput").ap()
    x0_d = nc.dram_tensor("x0b", [128, KU * BLOC], BF16,
                          kind="ExternalInput").ap()
    id_d = nc.dram_tensor("id128", [128, 128], BF16, kind="ExternalInput").ap()
    if with_bias:
        bias_d = nc.dram_tensor("biasT", [1, U], BF16,
                                kind="ExternalInput").ap()
    out_d = nc.dram_tensor("out", [128, t_steps * KU * BLOC], BF16,
                           kind="ExternalOutput").ap()

    SW = KU * BLOC            # 64: state width (j*8 + b)

    with tile.TileContext(nc) as tc, ExitStack() as ctx:
        const = ctx.enter_context(tc.tile_pool(name="const", bufs=1))
        # resident tensors. W tiles are split per (phase, kc) so the scan's
        # first steps only wait on the W_inv DMAs.
        w_sb = [[const.tile([128, U], BF16, tag=f"w{p}_{k}", name=f"w{p}_{k}")
                 for k in range(KU)] for p in range(2)]
        ra_sb = [const.tile([128, U], BF16, tag=f"ra{k}", name=f"ra{k}") for k in range(KF)]
        h_sb = [const.tile([128, TCH * SW], BF16, tag=f"h{c}", name=f"h{c}")
                for c in range(n_ch)]
        x0_sb = const.tile([128, SW], BF16, tag="x0")
        id_sb = const.tile([128, 128], BF16, tag="id")
        if with_bias:
            bias_sb = const.tile([1, U], BF16, tag="bias")
            ones_sb = const.tile([1, 512], BF16, tag="ones")

        # prologue DMAs (all on the SP queue; issue order matters: scan
        # step 0 needs R+x chunk 0 (via phase A) and the W_inv tiles).
        for k in range(KF):
            nc.sync.dma_start(ra_sb[k][:, :], R_d[k * 128:(k + 1) * 128, :])
        nc.sync.dma_start(x0_sb[:, :], x0_d[:, :])
        nc.sync.dma_start(id_sb[:, :], id_d[:, :])
        if with_bias:
            nc.sync.dma_start(bias_sb[:, :], bias_d[:, :])
            nc.vector.memset(ones_sb[:, :], 1.0)

        # ---------------- phase A: hT = (x @ R + bias)^T -> bf16 SBUF
        # h_sb[c][p, tl*SW + j*8 + b] = h[t = c*TCH + tl, u = j*128 + p, b]
        xpool = ctx.enter_context(tc.tile_pool(name="xstage", bufs=2))
        papool = ctx.enter_context(
            tc.tile_pool(name="psum_pa", bufs=2, space="PSUM")
        )
        CC = 2                   # 256-col moving chunks per pa matmul
        CW = 512 // CC
        pa_state = {}

        def pa_dma_item(c):
            def run():
                xa = xpool.tile([128, KF, 512], BF16, tag="xa", name="xa")
                for k in range(KF):
                    nc.sync.dma_start(
                        xa[:, k, :],
                        xT_d[k * 128:(k + 1) * 128, c * 512:(c + 1) * 512],
                    )
                pa_state[c] = xa
            return run

        def pa_mm_item(c, j, k, cc):
            def run():
                if k == 0 and cc == 0:
                    pa_state["ps"] = papool.tile([128, 512], F32, tag="psA", name="psA")
                last = (k == KF - 1 and cc == CC - 1) and not with_bias
                nc.tensor.matmul(
                    pa_state["ps"][:, cc * CW:(cc + 1) * CW],
                    ra_sb[k][:, j * 128:(j + 1) * 128],
                    pa_state[c][:, k, cc * CW:(cc + 1) * CW],
                    start=(k == 0 and cc == 0),
                    stop=last,
                )
                if with_bias and k == KF - 1:
                    nc.tensor.matmul(
                        pa_state["ps"][:, cc * CW:(cc + 1) * CW],
                        bias_sb[0:1, j * 128:(j + 1) * 128],
                        ones_sb[0:1, cc * CW:(cc + 1) * CW],
                        start=False,
                        stop=(cc == CC - 1),
                    )
            return run

        def pa_copy_item(c, j):
            def run():
                # psum [128, 512] = [tl 64, b 8] -> h_sb[c] cols tl*64+j*8+b
                dst = h_sb[c].rearrange("p (t w) -> p t w", w=SW)[:, :, j * 8:(j + 1) * 8]
                src = pa_state["ps"].rearrange("p (t b) -> p t b", b=8)[:, :, :]
                nc.vector.tensor_copy(dst, src)
            return run

        pa_items = []
        for c in range(n_ch):
            if c == 0:
                pa_items.append(pa_dma_item(0))
            for j in range(KU):
                # prefetch next chunk's x at the midpoint of this chunk
                if j == KU // 2 and c + 1 < n_ch:
                    pa_items.append(pa_dma_item(c + 1))
                for k in range(KF):
                    for cc in range(CC):
                        pa_items.append(pa_mm_item(c, j, k, cc))
                pa_items.append(pa_copy_item(c, j))
        pa_pos = [0]

        def pa_pull(limit):
            done = 0
            while done < limit and pa_pos[0] < len(pa_items):
                pa_items[pa_pos[0]]()
                pa_pos[0] += 1
                done += 1

        # bootstrap: chunks 0 and 1 fully (DMAs + mms + copies)
        boot_items = 1 + 2 * (KU * (KF * CC + 1) + 1) - 1
        pa_pull(boot_items)

        # W tiles after the bootstrap x DMAs so step-0's deps land first
        for p in range(2):
            for k in range(KU):
                nc.sync.dma_start(
                    w_sb[p][k][:, :], W2_d[p, k * 128:(k + 1) * 128, :]
                )

        # ---------------- phase B: the scan
        mmpool = ctx.enter_context(
            tc.tile_pool(name="psum_mm", bufs=2, space="PSUM")
        )
        rpool = ctx.enter_context(tc.tile_pool(name="ring", bufs=2))

        ring = None
        prev_tile, prev_off = x0_sb, 0
        for t in range(t_steps):
            c, tl = t // TCH, t % TCH
            p = 1 if (t // TCH) % 2 == 1 else 0
            if t % RING == 0:
                ring = rpool.tile([128, RING * SW], BF16, tag="ring", name="ring")
            ps = mmpool.tile([128, SW], F32, tag="ps", name="ps")
            # h first: no dependency on the previous state, so it fills the
            # PE idle window while tanh(t-1) is still in flight.
            nc.tensor.matmul(
                ps[:, :],
                id_sb[:, :],
                h_sb[c][:, tl * SW:(tl + 1) * SW],
                start=True,
                stop=False,
            )
            for j in range(KU):
                for k in range(KU):
                    nc.tensor.matmul(
                        ps[:, j * 8:(j + 1) * 8],
                        w_sb[p][k][:, j * 128:(j + 1) * 128],
                        prev_tile[:, prev_off + k * 8:prev_off + (k + 1) * 8],
                        start=False,
                        stop=(j == KU - 1 and k == KU - 1),
                    )
            # phase-A filler work rides in the PE bubble created by the
            # tanh wait (emitted after this step's mms, before the next's).
            pa_pull(1)
            so = (t % RING) * SW
            nc.scalar.activation(
                ring[:, so:so + SW],
                ps[:, :],
                mybir.ActivationFunctionType.Tanh,
            )
            prev_tile, prev_off = ring, so
            if t % RING == RING - 1:
                nc.sync.dma_start(
                    out_d[:, (t - RING + 1) * SW:(t + 1) * SW], ring[:, :]
                )
        pa_pull(len(pa_items))      # drain any leftovers (shouldn't exist)

    legalize_waits(nc)
    return nc


# -------------------------------------------------------------- host driver
_CACHE = {}


def _get_nc(t_steps, with_bias=False):
    key = (t_steps, with_bias)
    if key not in _CACHE:
        _CACHE[key] = build_kernel(t_steps, with_bias)
    return _CACHE[key]


def kernel(inputs, R, W, bias, x0, t_steps=None, n_cores=NCORES, trace=False,
           trace_kw=None):
    import ml_dtypes
    bf16 = ml_dtypes.bfloat16

    t_steps = t_steps or inputs.shape[1]
    inputs = np.ascontiguousarray(inputs, dtype=np.float32)
    R = np.asarray(R, dtype=np.float32)
    W = np.asarray(W, dtype=np.float32)
    bias = np.asarray(bias, dtype=np.float32)
    x0 = np.asarray(x0, dtype=np.float32)

    W_inv = np.linalg.inv(W)
    W2 = np.stack([W_inv, W]).astype(bf16)              # phase 0 = W_inv
    Rb = R.astype(bf16)
    # x0b[p, j*8+b] = x0[j*128+p]
    x0b = np.repeat(
        x0.reshape(KU, 128).T[:, :, None], BLOC, axis=2
    ).reshape(128, KU * BLOC).astype(bf16)
    id128 = np.eye(128, dtype=np.float32).astype(bf16)
    with_bias = bool(np.any(bias))
    biasT = None
    if with_bias:
        biasT = np.ascontiguousarray(bias.reshape(1, U)).astype(bf16)

    in_maps = []
    for c in range(n_cores):
        xc = inputs[c * BLOC:(c + 1) * BLOC, :t_steps, :]   # [BLOC, t, F]
        xT = np.ascontiguousarray(
            xc.transpose(2, 1, 0).reshape(F, BLOC * t_steps)
        ).astype(bf16)
        m = {"xT": xT, "R": Rb, "W2": W2, "x0b": x0b, "id128": id128}
        if with_bias:
            m["biasT"] = biasT
        in_maps.append(m)

    nc = _get_nc(t_steps, with_bias)
    try:
        res = run_bass_kernel_spmd(
            nc, in_maps, core_ids=list(range(n_cores)), trace=trace,
            **(trace_kw or {}),
        )
    except Exception:
        # transient device wedges (NRT_EXEC_UNIT_UNRECOVERABLE) usually
        # clear on a retry
        res = run_bass_kernel_spmd(
            nc, in_maps, core_ids=list(range(n_cores)), trace=trace,
            **(trace_kw or {}),
        )
    kernel.last_result = res
    kernel.last_nc = nc
    # assemble [T, B, U]: per-core out is [128, t*64] bf16 transposed state
    full = np.empty((t_steps, n_cores * BLOC, U), np.float32)
    for c in range(n_cores):
        arr = np.asarray(res.results[c]["out"])          # [128, t*64] bf16
        full[:, c * BLOC:(c + 1) * BLOC, :] = (
            arr.reshape(128, t_steps, KU, BLOC)
            .transpose(1, 3, 2, 0)
            .reshape(t_steps, BLOC, U)
            .astype(np.float32)
        )
    return full


# revision 27
# speedup vs baseline: 3.8826x; 1.0265x over previous
"""Trainium2 Bass kernel for nn_InverseRecurrentLayer.

Reference computation:
    W_inv = inv(W)
    h[t] = inputs[:, t, :] @ R + bias      # [B, U]  (bias folded into h)
    s_{t+1} = tanh(h[t] + s_t @ Wt),  Wt = W if (t//64)%2==1 else W_inv
    output = states [T, B, U]

Shapes: B=64, T=512, F=512, U=1024. fp32 in/out. Data-parallel over
batch: 8 cores x B_loc=8.

Per-core plan (v5 — W-stationary scan):
  The v4 kernel streamed all of W through the PE as the *moving* tensor
  every step (8192 moving cols/step -> 3.4us/step floor). v5 flips the
  matmul: W chunks [128,128] are the *stationary* operand and the
  transposed state sT chunk [128, 8] is the moving operand, so a full
  state update streams only 8kc x 8uo x 8 cols = 512 moving columns.
  With bf16 operands (1 cycle/col at any size) the per-step PE time is
  ~0.25us and the critical path becomes the PE->ACT->PE latency chain.

  State layout: one tile s[128, 64], col = j*8 + b for u-chunk j, batch
  b; partition = u % 128. A step is:
    - 1 identity matmul: psum[128,64]  = I128^T @ h_t[128,64] (start)
    - 64 matmuls:        psum[:, j*8:] += Wc[kc,j]^T @ s_prev[:, kc*8:]
    - 1 ACT tanh:        s_new[128,64](bf16) = tanh(psum)
  s_new is written into a 16-step ring; one DMA per 16 steps stores the
  ring to DRAM (bf16; host converts/untransposes).

  Phase A (h = x @ R + bias, transposed layout hT[u, (t,j,b)]) runs as
  bf16 matmuls (256-col moving chunks) interleaved one item per step
  into the scan's PE idle window, with DVE draining psum-> resident
  h_sb tiles (one tile per 64-step chunk to avoid whole-tile hazards
  with the scan's h reads).

This environment's walrus encodes at most ONE sync-wait command per
instruction; legalize_waits() hoists extra waits onto InstNoOp carriers,
and the Tile exit barrier is patched to sem-only barriers.
"""
import sys

sys.path.insert(0, "/opt/trn_rl_repo")

import numpy as np
from contextlib import ExitStack

import concourse.bass as bass
import concourse.mybir as mybir
import concourse.tile as tile
from concourse.bass_utils import run_bass_kernel_spmd

# ---------------------------------------------------------------- constants
B, T, F, U = 64, 512, 512, 1024
NCORES = 8
BLOC = B // NCORES          # 8 batch rows per core
KF = F // 128               # 4 f-chunks for the projection
KU = U // 128               # 8 u-chunks
TCH = 32                    # steps per h chunk
RING = 16                   # steps per output ring/DMA
F32 = mybir.dt.float32
F32R = mybir.dt.float32r
BF16 = mybir.dt.bfloat16

# ------------------------------------------------- walrus wait legalization


def _patched_drain_and_barrier(self, tick_clock, wait_clock):
    drain_inst = self.nc.sync.drain()
    wait_clock.add_sem_waits(
        drain_inst.ins, tile.ScopedClock({None: tick_clock.global_clock})
    )
    ow = list(drain_inst.ins.sync_info.on_wait or [])
    if len(ow) > 1:
        drain_inst.ins.sync_info.on_wait = ow[:1]
        for w in ow[1:]:
            d2 = self.nc.sync.drain()
            d2.ins.sync_info = mybir.SyncInfo(on_wait=[w], on_update=[])
    self.nc.all_engine_barrier(sem_only=True)
    popped = self.nc._tile_sem_poison_stack.pop()
    assert popped is self._sem_poison
    self.nc.clear_and_free_semaphores(list(self.sems.allocated().values()))
    self.nc.all_engine_barrier(sem_only=True)


tile.TileContext._drain_and_barrier = _patched_drain_and_barrier


def legalize_waits(nc):
    """Split multi-wait instructions: keep 1 wait, hoist the rest onto
    InstNoOp carriers inserted just before, on the same engine."""
    n = 0
    for fn in nc.m.functions:
        for blk in fn.blocks:
            out = []
            for inst in blk.instructions:
                si = inst.sync_info
                if si is not None and si.on_wait and len(si.on_wait) > 1:
                    waits = list(si.on_wait)
                    for w in waits[:-1]:
                        n += 1
                        nop = mybir.InstNoOp(
                            name=f"waitcar-{n}-{inst.name}",
                            engine=inst.engine,
                            ins=[],
                            outs=[],
                            sync_info=mybir.SyncInfo(on_wait=[w], on_update=[]),
                        )
                        nc.register_instruction(nop)
                        out.append(nop)
                    si.on_wait = waits[-1:]
                out.append(inst)
            blk.instructions[:] = out
    return n


# ------------------------------------------------------------ device kernel


def build_kernel(t_steps=T, with_bias=False):
    assert t_steps % TCH == 0 and t_steps % RING == 0
    nc = bass.Bass("TRN2", target_bir_lowering=False, debug=False)
    tb = BLOC * t_steps
    n_ch = (t_steps + TCH - 1) // TCH       # h chunks (64 steps each)

    xT_d = nc.dram_tensor("xT", [F, tb], BF16, kind="ExternalInput").ap()
    R_d = nc.dram_tensor("R", [F, U], BF16, kind="ExternalInput").ap()
    W2_d = nc.dram_tensor("W2", [2, U, U], BF16, kind="ExternalInput").ap()
    x0_d = nc.dram_tensor("x0b", [128, KU * BLOC], BF16,
                          kind="ExternalInput").ap()
    id_d = nc.dram_tensor("id128", [128, 128], BF16, kind="ExternalInput").ap()
    if with_bias:
        bias_d = nc.dram_tensor("biasT", [1, U], BF16,
                                kind="ExternalInput").ap()
    out_d = nc.dram_tensor("out", [128, t_steps * KU * BLOC], BF16,
                           kind="ExternalOutput").ap()

    SW = KU * BLOC            # 64: state width (j*8 + b)

    with tile.TileContext(nc) as tc, ExitStack() as ctx:
        const = ctx.enter_context(tc.tile_pool(name="const", bufs=1))
        # resident tensors. W tiles are split per (phase, kc) so the scan's
        # first steps only wait on the W_inv DMAs.
        w0_sb = [const.tile([128, 4 * U], BF16, tag=f"w0_{h}", name=f"w0_{h}")
                 for h in range(2)]
        w1_sb = [const.tile([128, U], BF16, tag=f"w1_{k}", name=f"w1_{k}")
                 for k in range(KU)]

        def wslice(p, k, j):
            if p == 0:
                return w0_sb[k // 4][:, (k % 4) * U + j * 128:
                                     (k % 4) * U + (j + 1) * 128]
            return w1_sb[k][:, j * 128:(j + 1) * 128]
        ra_sb = [const.tile([128, U], BF16, tag=f"ra{k}", name=f"ra{k}") for k in range(KF)]
        h_sb = [const.tile([128, TCH * SW], BF16, tag=f"h{c}", name=f"h{c}")
                for c in range(n_ch)]
        x0_sb = const.tile([128, SW], BF16, tag="x0")
        id_sb = const.tile([128, 128], BF16, tag="id")
        if with_bias:
            bias_sb = const.tile([1, U], BF16, tag="bias")
            ones_sb = const.tile([1, 512], BF16, tag="ones")

        # prologue DMAs: step-0 needs x0 (PE warmup), R + x chunk 0 (the
        # h projection) and all of W_inv. W_inv goes as two fat DMAs (one
        # per HWDGE queue) behind the small tensors; the phase-1 W tiles
        # (not needed until step 64, ~70us in) are issued mid-scan from the
        # phase-A item stream so they don't contend for the DMA pipe here.
        W2v = W2_d.rearrange("q (k p) u -> q p k u", p=128)
        nc.scalar.dma_start(x0_sb[:, :], x0_d[:, :])
        for k in (0, 1):
            nc.scalar.dma_start(ra_sb[k][:, :], R_d[k * 128:(k + 1) * 128, :])
        sp_prologue = [
            lambda: nc.sync.dma_start(ra_sb[2][:, :], R_d[256:384, :]),
            lambda: nc.sync.dma_start(ra_sb[3][:, :], R_d[384:512, :]),
            lambda: nc.sync.dma_start(id_sb[:, :], id_d[:, :]),
            lambda: nc.sync.dma_start(
                w0_sb[0].rearrange("p (k u) -> p k u", u=U), W2v[0, :, 0:4, :]),
        ]
        nc.scalar.dma_start(
            w0_sb[1].rearrange("p (k u) -> p k u", u=U), W2v[0, :, 4:8, :])
        if with_bias:
            nc.scalar.dma_start(bias_sb[:, :], bias_d[:, :])
            nc.vector.memset(ones_sb[:, :], 1.0)

        # ---------------- phase A: hT = (x @ R + bias)^T -> bf16 SBUF
        # h_sb[c][p, tl*SW + j*8 + b] = h[t = c*TCH + tl, u = j*128 + p, b]
        CW = TCH * BLOC          # 256 moving cols per pa matmul
        xpool = ctx.enter_context(tc.tile_pool(name="xstage", bufs=2))
        papool = ctx.enter_context(
            tc.tile_pool(name="psum_pa", bufs=2, space="PSUM")
        )
        pa_state = {}

        xT_v = xT_d.rearrange("(k p) t -> p k t", k=KF)

        def pa_dma_item(c):
            def run():
                xa = xpool.tile([128, KF, CW], BF16, tag="xa", name="xa")
                nc.sync.dma_start(xa[:, :, :], xT_v[:, :, c * CW:(c + 1) * CW])
                pa_state[c] = xa
            return run

        def pa_mm_item(c, j, k):
            def run():
                if k == 0:
                    pa_state["ps"] = papool.tile([128, CW], F32, tag="psA", name="psA")
                last = (k == KF - 1) and not with_bias
                nc.tensor.matmul(
                    pa_state["ps"][:, :],
                    ra_sb[k][:, j * 128:(j + 1) * 128],
                    pa_state[c][:, k, :],
                    start=(k == 0),
                    stop=last,
                )
                if with_bias and k == KF - 1:
                    nc.tensor.matmul(
                        pa_state["ps"][:, :],
                        bias_sb[0:1, j * 128:(j + 1) * 128],
                        ones_sb[0:1, :CW],
                        start=False,
                        stop=True,
                    )
            return run

        def pa_copy_item(c, j):
            def run():
                # psum [128, CW] = [tl, b] -> h_sb[c] cols tl*64 + j*8 + b
                dst = h_sb[c].rearrange("p (t w) -> p t w", w=SW)[:, :, j * 8:(j + 1) * 8]
                src = pa_state["ps"].rearrange("p (t b) -> p t b", b=8)[:, :, :]
                nc.vector.tensor_copy(dst, src)
            return run

        pa_items = []
        for c in range(n_ch):
            if c == 0:
                pa_items.append(pa_dma_item(0))
            for j in range(KU):
                # prefetch next chunk's x at the midpoint of this chunk
                if j == KU // 2 and c + 1 < n_ch:
                    pa_items.append(pa_dma_item(c + 1))
                for k in range(KF):
                    pa_items.append(pa_mm_item(c, j, k))
                pa_items.append(pa_copy_item(c, j))
        pa_pos = [0]

        def pa_pull(limit):
            done = 0
            while done < limit and pa_pos[0] < len(pa_items):
                pa_items[pa_pos[0]]()
                pa_pos[0] += 1
                done += 1

        # xa0 fat DMA first on SP, then the remaining W_inv tiles
        pa_pull(1)
        for f in sp_prologue:
            f()

        # bootstrap: chunk 0 only (the scan starts after ~32 pa matmuls)
        boot_items = KU * (KF + 1) + 1
        pa_pull(boot_items)

        # phase-1 W tiles: on SP *after* the prologue/xa DMAs. SP is
        # in-order, so these 8 transfers hit the DMA pipe only once the
        # step-0 dependencies are through (they're not needed until ~70us).
        for k in range(KU):
            nc.sync.dma_start(w1_sb[k][:, :], W2_d[1, k * 128:(k + 1) * 128, :])


        # ---------------- phase B: the scan
        mmpool = ctx.enter_context(
            tc.tile_pool(name="psum_mm", bufs=2, space="PSUM")
        )
        rpool = ctx.enter_context(tc.tile_pool(name="ring", bufs=2))

        ring = None
        ring_next = [None]
        prev_tile, prev_off = x0_sb, 0
        for t in range(t_steps):
            c, tl = t // TCH, t % TCH
            p = 1 if (t // 64) % 2 == 1 else 0      # INVERT_INDEX = 64
            if t % RING == 0:
                if ring_next[0] is not None:
                    ring = ring_next[0]
                    ring_next[0] = None
                else:
                    ring = rpool.tile([128, RING * SW], BF16, tag="ring",
                                      name="ring")
            ps = mmpool.tile([128, SW], F32, tag="ps", name="ps")
            # h first: no dependency on the previous state, so it fills the
            # PE idle window while tanh(t-1) is still in flight.
            nc.tensor.matmul(
                ps[:, :],
                id_sb[:, :],
                h_sb[c][:, tl * SW:(tl + 1) * SW],
                start=True,
                stop=False,
            )
            for j in range(KU):
                for k in range(KU):
                    nc.tensor.matmul(
                        ps[:, j * 8:(j + 1) * 8],
                        wslice(p, k, j),
                        prev_tile[:, prev_off + k * 8:prev_off + (k + 1) * 8],
                        start=False,
                        stop=(j == KU - 1 and k == KU - 1),
                    )
            # phase-A filler work rides in the PE bubble created by the
            # tanh wait (emitted after this step's mms, before the next's).
            # Steps 0-7: no pulls (chunk-1 items would park the PE queue on
            # the late-landing xa1 DMA); catch up afterwards.
            pa_pull(0 if t < 8 else (3 if t < 24 else 2))
            so = (t % RING) * SW
            nc.scalar.activation(
                ring[:, so:so + SW],
                ps[:, :],
                mybir.ActivationFunctionType.Tanh,
            )
            prev_tile, prev_off = ring, so
            if t % RING == RING // 2 and t + RING < t_steps:
                # pre-allocate the next ring and absorb its buffer-recycle
                # WAR wait into an off-path ACT dummy write: without this
                # the first tanh of each window carries 2 sem waits and the
                # legalized waitcar NoOp parks the ACT sequencer (+78ns).
                rn = rpool.tile([128, RING * SW], BF16, tag="ring",
                                name="ring")
                nc.scalar.activation(
                    rn[:, 0:8], x0_sb[:, 0:8],
                    mybir.ActivationFunctionType.Copy,
                )
                ring_next[0] = rn
            if t == t_steps - 1 - RING // 2:
                # early-drain the first half of the final window so the
                # epilogue only waits on an 8-slot DMA (on SP: idle by now
                # and HWDGE issues faster than SWDGE)
                nc.sync.dma_start(
                    out_d[:, (t - RING // 2 + 1) * SW:(t + 1) * SW],
                    ring[:, :(RING // 2) * SW],
                )
            elif t == t_steps - 1:
                nc.sync.dma_start(
                    out_d[:, (t - RING // 2 + 1) * SW:(t + 1) * SW],
                    ring[:, (RING // 2) * SW:],
                )
            elif t % RING == RING - 1:
                nc.gpsimd.dma_start(
                    out_d[:, (t - RING + 1) * SW:(t + 1) * SW], ring[:, :]
                )
        pa_pull(len(pa_items))      # drain any leftovers (shouldn't exist)

    legalize_waits(nc)
    return nc


# -------------------------------------------------------------- host driver
_CACHE = {}


def _get_nc(t_steps, with_bias=False):
    key = (t_steps, with_bias)
    if key not in _CACHE:
        _CACHE[key] = build_kernel(t_steps, with_bias)
    return _CACHE[key]


def kernel(inputs, R, W, bias, x0, t_steps=None, n_cores=NCORES, trace=False,
           trace_kw=None):
    import ml_dtypes
    bf16 = ml_dtypes.bfloat16

    t_steps = t_steps or inputs.shape[1]
    inputs = np.ascontiguousarray(inputs, dtype=np.float32)
    R = np.asarray(R, dtype=np.float32)
    W = np.asarray(W, dtype=np.float32)
    bias = np.asarray(bias, dtype=np.float32)
    x0 = np.asarray(x0, dtype=np.float32)

    W_inv = np.linalg.inv(W)
    W2 = np.stack([W_inv, W]).astype(bf16)              # phase 0 = W_inv
    Rb = R.astype(bf16)
    # x0b[p, j*8+b] = x0[j*128+p]
    x0b = np.repeat(
        x0.reshape(KU, 128).T[:, :, None], BLOC, axis=2
    ).reshape(128, KU * BLOC).astype(bf16)
    id128 = np.eye(128, dtype=np.float32).astype(bf16)
    with_bias = bool(np.any(bias))
    biasT = None
    if with_bias:
        biasT = np.ascontiguousarray(bias.reshape(1, U)).astype(bf16)

    in_maps = []
    for c in range(n_cores):
        xc = inputs[c * BLOC:(c + 1) * BLOC, :t_steps, :]   # [BLOC, t, F]
        xT = np.ascontiguousarray(
            xc.transpose(2, 1, 0).reshape(F, BLOC * t_steps)
        ).astype(bf16)
        m = {"xT": xT, "R": Rb, "W2": W2, "x0b": x0b, "id128": id128}
        if with_bias:
            m["biasT"] = biasT
        in_maps.append(m)

    nc = _get_nc(t_steps, with_bias)
    try:
        res = run_bass_kernel_spmd(
            nc, in_maps, core_ids=list(range(n_cores)), trace=trace,
            **(trace_kw or {}),
        )
    except Exception:
        # transient device wedges (NRT_EXEC_UNIT_UNRECOVERABLE) usually
        # clear on a retry
        res = run_bass_kernel_spmd(
            nc, in_maps, core_ids=list(range(n_cores)), trace=trace,
            **(trace_kw or {}),
        )
    kernel.last_result = res
    kernel.last_nc = nc
    # assemble [T, B, U]: per-core out is [128, t*64] bf16 transposed state
    full = np.empty((t_steps, n_cores * BLOC, U), np.float32)
    for c in range(n_cores):
        arr = np.asarray(res.results[c]["out"])          # [128, t*64] bf16
        full[:, c * BLOC:(c + 1) * BLOC, :] = (
            arr.reshape(128, t_steps, KU, BLOC)
            .transpose(1, 3, 2, 0)
            .reshape(t_steps, BLOC, U)
            .astype(np.float32)
        )
    return full


# revision 29
# speedup vs baseline: 13.9935x; 3.6041x over previous
"""Trainium2 Bass kernel for nn_InverseRecurrentLayer.

Reference computation:
    W_inv = inv(W)
    h[t] = inputs[:, t, :] @ R + bias      # [B, U]  (bias folded into h)
    s_{t+1} = tanh(h[t] + s_t @ Wt),  Wt = W if (t//64)%2==1 else W_inv
    output = states [T, B, U]

Shapes: B=64, T=512, F=512, U=1024. fp32 in/out. Data-parallel over
batch: 8 cores x B_loc=8.

Per-core plan (v5 — W-stationary scan):
  The v4 kernel streamed all of W through the PE as the *moving* tensor
  every step (8192 moving cols/step -> 3.4us/step floor). v5 flips the
  matmul: W chunks [128,128] are the *stationary* operand and the
  transposed state sT chunk [128, 8] is the moving operand, so a full
  state update streams only 8kc x 8uo x 8 cols = 512 moving columns.
  With bf16 operands (1 cycle/col at any size) the per-step PE time is
  ~0.25us and the critical path becomes the PE->ACT->PE latency chain.

  State layout: one tile s[128, 64], col = j*8 + b for u-chunk j, batch
  b; partition = u % 128. A step is:
    - 1 identity matmul: psum[128,64]  = I128^T @ h_t[128,64] (start)
    - 64 matmuls:        psum[:, j*8:] += Wc[kc,j]^T @ s_prev[:, kc*8:]
    - 1 ACT tanh:        s_new[128,64](bf16) = tanh(psum)
  s_new is written into a 16-step ring; one DMA per 16 steps stores the
  ring to DRAM (bf16; host converts/untransposes).

  Phase A (h = x @ R + bias, transposed layout hT[u, (t,j,b)]) runs as
  bf16 matmuls (256-col moving chunks) interleaved one item per step
  into the scan's PE idle window, with DVE draining psum-> resident
  h_sb tiles (one tile per 64-step chunk to avoid whole-tile hazards
  with the scan's h reads).

This environment's walrus encodes at most ONE sync-wait command per
instruction; legalize_waits() hoists extra waits onto InstNoOp carriers,
and the Tile exit barrier is patched to sem-only barriers.
"""
import sys

sys.path.insert(0, "/opt/trn_rl_repo")

import numpy as np
from contextlib import ExitStack

import concourse.bass as bass
import concourse.mybir as mybir
import concourse.tile as tile
from concourse.bass_utils import run_bass_kernel_spmd

# ---------------------------------------------------------------- constants
B, T, F, U = 64, 512, 512, 1024
NCORES = 8
BLOC = B // NCORES          # 8 batch rows per core
KF = F // 128               # 4 f-chunks for the projection
KU = U // 128               # 8 u-chunks
TCH = 32                    # steps per h chunk
RING = 16                   # steps per output ring/DMA
F32 = mybir.dt.float32
F32R = mybir.dt.float32r
BF16 = mybir.dt.bfloat16

# ------------------------------------------------- walrus wait legalization


def _patched_drain_and_barrier(self, tick_clock, wait_clock):
    drain_inst = self.nc.sync.drain()
    wait_clock.add_sem_waits(
        drain_inst.ins, tile.ScopedClock({None: tick_clock.global_clock})
    )
    ow = list(drain_inst.ins.sync_info.on_wait or [])
    if len(ow) > 1:
        drain_inst.ins.sync_info.on_wait = ow[:1]
        for w in ow[1:]:
            d2 = self.nc.sync.drain()
            d2.ins.sync_info = mybir.SyncInfo(on_wait=[w], on_update=[])
    self.nc.all_engine_barrier(sem_only=True)
    popped = self.nc._tile_sem_poison_stack.pop()
    assert popped is self._sem_poison
    self.nc.clear_and_free_semaphores(list(self.sems.allocated().values()))
    self.nc.all_engine_barrier(sem_only=True)


tile.TileContext._drain_and_barrier = _patched_drain_and_barrier


def legalize_waits(nc):
    """Split multi-wait instructions: keep 1 wait, hoist the rest onto
    InstNoOp carriers inserted just before, on the same engine."""
    n = 0
    for fn in nc.m.functions:
        for blk in fn.blocks:
            out = []
            for inst in blk.instructions:
                si = inst.sync_info
                if si is not None and si.on_wait and len(si.on_wait) > 1:
                    waits = list(si.on_wait)
                    for w in waits[:-1]:
                        n += 1
                        nop = mybir.InstNoOp(
                            name=f"waitcar-{n}-{inst.name}",
                            engine=inst.engine,
                            ins=[],
                            outs=[],
                            sync_info=mybir.SyncInfo(on_wait=[w], on_update=[]),
                        )
                        nc.register_instruction(nop)
                        out.append(nop)
                    si.on_wait = waits[-1:]
                out.append(inst)
            blk.instructions[:] = out
    return n


# ------------------------------------------------------------ device kernel


def build_kernel(t_steps=T, with_bias=False):
    assert t_steps % TCH == 0 and t_steps % RING == 0
    nc = bass.Bass("TRN2", target_bir_lowering=False, debug=False)
    tb = BLOC * t_steps
    n_ch = (t_steps + TCH - 1) // TCH       # h chunks (64 steps each)

    xT_d = nc.dram_tensor("xT", [F, tb], BF16, kind="ExternalInput").ap()
    R_d = nc.dram_tensor("R", [F, U], BF16, kind="ExternalInput").ap()
    W2_d = nc.dram_tensor("W2", [2, U, U], BF16, kind="ExternalInput").ap()
    x0_d = nc.dram_tensor("x0b", [128, KU * BLOC], BF16,
                          kind="ExternalInput").ap()
    id_d = nc.dram_tensor("id128", [128, 128], BF16, kind="ExternalInput").ap()
    if with_bias:
        bias_d = nc.dram_tensor("biasT", [1, U], BF16,
                                kind="ExternalInput").ap()
    out_d = nc.dram_tensor("out", [128, t_steps * KU * BLOC], BF16,
                           kind="ExternalOutput").ap()

    SW = KU * BLOC            # 64: state width (j*8 + b)

    with tile.TileContext(nc) as tc, ExitStack() as ctx:
        const = ctx.enter_context(tc.tile_pool(name="const", bufs=1))
        # resident tensors. W tiles are split per (phase, kc) so the scan's
        # first steps only wait on the W_inv DMAs.
        w0_sb = [const.tile([128, 4 * U], BF16, tag=f"w0_{h}", name=f"w0_{h}")
                 for h in range(2)]
        w1_sb = [const.tile([128, U], BF16, tag=f"w1_{k}", name=f"w1_{k}")
                 for k in range(KU)]

        def wslice(p, k, j):
            if p == 0:
                return w0_sb[k // 4][:, (k % 4) * U + j * 128:
                                     (k % 4) * U + (j + 1) * 128]
            return w1_sb[k][:, j * 128:(j + 1) * 128]
        ra_sb = [const.tile([128, U], BF16, tag=f"ra{k}", name=f"ra{k}") for k in range(KF)]
        h_sb = [const.tile([128, TCH * SW], BF16, tag=f"h{c}", name=f"h{c}")
                for c in range(n_ch)]
        x0_sb = const.tile([128, SW], BF16, tag="x0")
        id_sb = const.tile([128, 128], BF16, tag="id")
        if with_bias:
            bias_sb = const.tile([1, U], BF16, tag="bias")
            ones_sb = const.tile([1, 512], BF16, tag="ones")

        # prologue DMAs: step-0 needs x0 (PE warmup), R + x chunk 0 (the
        # h projection) and all of W_inv. W_inv goes as two fat DMAs (one
        # per HWDGE queue) behind the small tensors; the phase-1 W tiles
        # (not needed until step 64, ~70us in) are issued mid-scan from the
        # phase-A item stream so they don't contend for the DMA pipe here.
        W2v = W2_d.rearrange("q (k p) u -> q p k u", p=128)
        nc.scalar.dma_start(x0_sb[:, :], x0_d[:, :])
        for k in (0, 1):
            nc.scalar.dma_start(ra_sb[k][:, :], R_d[k * 128:(k + 1) * 128, :])
        sp_prologue = [
            lambda: nc.sync.dma_start(ra_sb[2][:, :], R_d[256:384, :]),
            lambda: nc.sync.dma_start(ra_sb[3][:, :], R_d[384:512, :]),
            lambda: nc.sync.dma_start(id_sb[:, :], id_d[:, :]),
            lambda: nc.sync.dma_start(
                w0_sb[0].rearrange("p (k u) -> p k u", u=U), W2v[0, :, 0:4, :]),
        ]
        nc.scalar.dma_start(
            w0_sb[1].rearrange("p (k u) -> p k u", u=U), W2v[0, :, 4:8, :])
        if with_bias:
            nc.scalar.dma_start(bias_sb[:, :], bias_d[:, :])
            nc.vector.memset(ones_sb[:, :], 1.0)

        # ---------------- phase A: hT = (x @ R + bias)^T -> bf16 SBUF
        # h_sb[c][p, tl*SW + j*8 + b] = h[t = c*TCH + tl, u = j*128 + p, b]
        CW = TCH * BLOC          # 256 moving cols per pa matmul
        xpool = ctx.enter_context(tc.tile_pool(name="xstage", bufs=2))
        papool = ctx.enter_context(
            tc.tile_pool(name="psum_pa", bufs=2, space="PSUM")
        )
        pa_state = {}

        xT_v = xT_d.rearrange("(k p) t -> p k t", k=KF)

        def pa_dma_item(c):
            def run():
                xa = xpool.tile([128, KF, CW], BF16, tag="xa", name="xa")
                nc.sync.dma_start(xa[:, :, :], xT_v[:, :, c * CW:(c + 1) * CW])
                pa_state[c] = xa
            return run

        def pa_mm_item(c, j, k):
            def run():
                if k == 0:
                    pa_state["ps"] = papool.tile([128, CW], F32, tag="psA", name="psA")
                last = (k == KF - 1) and not with_bias
                nc.tensor.matmul(
                    pa_state["ps"][:, :],
                    ra_sb[k][:, j * 128:(j + 1) * 128],
                    pa_state[c][:, k, :],
                    start=(k == 0),
                    stop=last,
                )
                if with_bias and k == KF - 1:
                    nc.tensor.matmul(
                        pa_state["ps"][:, :],
                        bias_sb[0:1, j * 128:(j + 1) * 128],
                        ones_sb[0:1, :CW],
                        start=False,
                        stop=True,
                    )
            return run

        def pa_copy_item(c, j):
            def run():
                # psum [128, CW] = [tl, b] -> h_sb[c] cols tl*64 + j*8 + b
                dst = h_sb[c].rearrange("p (t w) -> p t w", w=SW)[:, :, j * 8:(j + 1) * 8]
                src = pa_state["ps"].rearrange("p (t b) -> p t b", b=8)[:, :, :]
                nc.vector.tensor_copy(dst, src)
            return run

        pa_items = []
        for c in range(n_ch):
            if c == 0:
                pa_items.append(pa_dma_item(0))
            for j in range(KU):
                # prefetch next chunk's x at the midpoint of this chunk
                if j == KU // 2 and c + 1 < n_ch:
                    pa_items.append(pa_dma_item(c + 1))
                for k in range(KF):
                    pa_items.append(pa_mm_item(c, j, k))
                pa_items.append(pa_copy_item(c, j))
        pa_pos = [0]

        def pa_pull(limit):
            done = 0
            while done < limit and pa_pos[0] < len(pa_items):
                pa_items[pa_pos[0]]()
                pa_pos[0] += 1
                done += 1

        # xa0 fat DMA first on SP, then the remaining W_inv tiles
        pa_pull(1)
        for f in sp_prologue:
            f()

        # bootstrap: chunk 0 only (the scan starts after ~32 pa matmuls)
        boot_items = KU * (KF + 1) + 1
        pa_pull(boot_items)

        # phase-1 W tiles: on SP *after* the prologue/xa DMAs. SP is
        # in-order, so these 8 transfers hit the DMA pipe only once the
        # step-0 dependencies are through (they're not needed until ~70us).
        for k in range(KU):
            nc.sync.dma_start(w1_sb[k][:, :], W2_d[1, k * 128:(k + 1) * 128, :])


        # ---------------- phase B: the scan
        mmpool = ctx.enter_context(
            tc.tile_pool(name="psum_mm", bufs=2, space="PSUM")
        )
        rpool = ctx.enter_context(tc.tile_pool(name="ring", bufs=2))

        ring = None
        ring_next = [None]
        prev_tile, prev_off = x0_sb, 0
        for t in range(t_steps):
            c, tl = t // TCH, t % TCH
            p = 1 if (t // 64) % 2 == 1 else 0      # INVERT_INDEX = 64
            if t % RING == 0:
                if ring_next[0] is not None:
                    ring = ring_next[0]
                    ring_next[0] = None
                else:
                    ring = rpool.tile([128, RING * SW], BF16, tag="ring",
                                      name="ring")
            ps = mmpool.tile([128, SW], F32, tag="ps", name="ps")
            # h first: no dependency on the previous state, so it fills the
            # PE idle window while tanh(t-1) is still in flight.
            nc.tensor.matmul(
                ps[:, :],
                id_sb[:, :],
                h_sb[c][:, tl * SW:(tl + 1) * SW],
                start=True,
                stop=False,
            )
            for j in range(KU):
                for k in range(KU):
                    nc.tensor.matmul(
                        ps[:, j * 8:(j + 1) * 8],
                        wslice(p, k, j),
                        prev_tile[:, prev_off + k * 8:prev_off + (k + 1) * 8],
                        start=False,
                        stop=(j == KU - 1 and k == KU - 1),
                    )
            # phase-A filler work rides in the PE bubble created by the
            # tanh wait (emitted after this step's mms, before the next's).
            # Steps 0-7: no pulls (chunk-1 items would park the PE queue on
            # the late-landing xa1 DMA); catch up afterwards.
            pa_pull(0 if t < 2 else (3 if t < 24 else 2))
            so = (t % RING) * SW
            nc.scalar.activation(
                ring[:, so:so + SW],
                ps[:, :],
                mybir.ActivationFunctionType.Tanh,
            )
            prev_tile, prev_off = ring, so
            if t % RING == RING // 2 and t + RING < t_steps:
                # pre-allocate the next ring and absorb its buffer-recycle
                # WAR wait into an off-path ACT dummy write: without this
                # the first tanh of each window carries 2 sem waits and the
                # legalized waitcar NoOp parks the ACT sequencer (+78ns).
                rn = rpool.tile([128, RING * SW], BF16, tag="ring",
                                name="ring")
                nc.scalar.activation(
                    rn[:, 0:8], x0_sb[:, 0:8],
                    mybir.ActivationFunctionType.Copy,
                )
                ring_next[0] = rn
            if t >= t_steps - RING and (
                t == t_steps - 9 or t == t_steps - 5 or t == t_steps - 1
            ) and t % RING != RING - 1 or (
                t == t_steps - 1 and t % RING == RING - 1
            ):
                # drain the final window in shrinking pieces (8/4/4 slots)
                # so the epilogue tail only waits on a 4-slot DMA (on SP:
                # idle by now and HWDGE issues faster than SWDGE)
                w0 = t_steps - RING              # final window start
                lo = {t_steps - 9: 0, t_steps - 5: 8, t_steps - 1: 12}[t]
                hi = {t_steps - 9: 8, t_steps - 5: 12, t_steps - 1: 16}[t]
                nc.sync.dma_start(
                    out_d[:, (w0 + lo) * SW:(w0 + hi) * SW],
                    ring[:, lo * SW:hi * SW],
                )
            elif t % RING == RING - 1:
                nc.gpsimd.dma_start(
                    out_d[:, (t - RING + 1) * SW:(t + 1) * SW], ring[:, :]
                )
        pa_pull(len(pa_items))      # drain any leftovers (shouldn't exist)

    legalize_waits(nc)
    return nc


# -------------------------------------------------------------- host driver
_CACHE = {}


def _get_nc(t_steps, with_bias=False):
    key = (t_steps, with_bias)
    if key not in _CACHE:
        _CACHE[key] = build_kernel(t_steps, with_bias)
    return _CACHE[key]


def kernel(inputs, R, W, bias, x0, t_steps=None, n_cores=NCORES, trace=False,
           trace_kw=None):
    import ml_dtypes
    bf16 = ml_dtypes.bfloat16

    t_steps = t_steps or inputs.shape[1]
    inputs = np.ascontiguousarray(inputs, dtype=np.float32)
    R = np.asarray(R, dtype=np.float32)
    W = np.asarray(W, dtype=np.float32)
    bias = np.asarray(bias, dtype=np.float32)
    x0 = np.asarray(x0, dtype=np.float32)

    W_inv = np.linalg.inv(W)
    W2 = np.stack([W_inv, W]).astype(bf16)              # phase 0 = W_inv
    Rb = R.astype(bf16)
    # x0b[p, j*8+b] = x0[j*128+p]
    x0b = np.repeat(
        x0.reshape(KU, 128).T[:, :, None], BLOC, axis=2
    ).reshape(128, KU * BLOC).astype(bf16)
    id128 = np.eye(128, dtype=np.float32).astype(bf16)
    with_bias = bool(np.any(bias))
    biasT = None
    if with_bias:
        biasT = np.ascontiguousarray(bias.reshape(1, U)).astype(bf16)

    in_maps = []
    for c in range(n_cores):
        xc = inputs[c * BLOC:(c + 1) * BLOC, :t_steps, :]   # [BLOC, t, F]
        xT = np.ascontiguousarray(
            xc.transpose(2, 1, 0).reshape(F, BLOC * t_steps)
        ).astype(bf16)
        m = {"xT": xT, "R": Rb, "W2": W2, "x0b": x0b, "id128": id128}
        if with_bias:
            m["biasT"] = biasT
        in_maps.append(m)

    nc = _get_nc(t_steps, with_bias)
    try:
        res = run_bass_kernel_spmd(
            nc, in_maps, core_ids=list(range(n_cores)), trace=trace,
            **(trace_kw or {}),
        )
    except Exception:
        # transient device wedges (NRT_EXEC_UNIT_UNRECOVERABLE) usually
        # clear on a retry
        res = run_bass_kernel_spmd(
            nc, in_maps, core_ids=list(range(n_cores)), trace=trace,
            **(trace_kw or {}),
        )
    kernel.last_result = res
    kernel.last_nc = nc
    # assemble [T, B, U]: per-core out is [128, t*64] bf16 transposed state
    full = np.empty((t_steps, n_cores * BLOC, U), np.float32)
    for c in range(n_cores):
        arr = np.asarray(res.results[c]["out"])          # [128, t*64] bf16
        full[:, c * BLOC:(c + 1) * BLOC, :] = (
            arr.reshape(128, t_steps, KU, BLOC)
            .transpose(1, 3, 2, 0)
            .reshape(t_steps, BLOC, U)
            .astype(np.float32)
        )
    return full
